# revision 34
# baseline (speedup 1.0000x reference)
"""AttentionBlock Trainium2 kernel (B=4, N=2048, C=1024, H=16, D=64, EMB=1024).

    se = emb @ W_emb.T + b_emb;  scale, shift = split(se, 2, -1)
    h  = LN(x) * (1+scale) + shift
    q,k,v = split(h @ W_proj.T) -> (B,H,N,D);  q = LN(q); k = LN(k)  (over D)
    o  = softmax(q k^T / sqrt(D)) v  -> (B,N,C)
    out = o + o @ W_out.T

Sharding: 8 cores; core c -> batch b=c//2, query-half j=c%2. The host rolls
the token axis per core so its query tokens are always tokens 0:1024
(attention is permutation-equivariant over key/value tokens), giving one
symmetric SPMD NEFF with no collectives. Each core computes the full-batch
preamble (se/h/k/v over all 2048 tokens), and q/attention/out-proj for its
1024 rows.

Dataflow is feature-major (channels on partitions) end to end; all matmul
operands are bf16 (PSUM accumulation stays fp32):
  - LayerNorm over channels == partition reduction -> ones-column matmuls;
    per-token (free-dim) stats broadcast across partitions by bouncing rows
    through DRAM.
  - q/k LN is folded in *before* the score matmul: kc = (k - mu_k)*rk/8 and
    qc = (q - mu_q)*rq are materialized in stacked [128, T] layouts (head
    pair on partition halves), so scores are two concurrent row-tiled K=64
    matmuls (tile rows 0-63 / 64-127) and the softmax exp needs no
    per-partition scale -> one ACTIVATE covers both heads' PSUM banks.
  - Per-head per-token mean/E[x^2] come from [P,2]-column reducer matmuls
    whose outputs land on adjacent partitions {0,1} for single-op row
    extraction.
  - Softmax denominators come free as a ones column appended to v; division
    is deferred: unnormalized o and the denominator rows are staged, then
    one batched reciprocal_approx_fast per head-quad rescales o_fm in place.
  - rstd/rk/rq use exp(-0.5*ln(var+eps)) so the scalar engine only ever
    needs the natural_log_exp table set (no table thrashing with softmax).
  - The residual is folded into the output projection: W_res = (I+W_out).T.
"""

import sys

sys.path.insert(0, "/opt/trn_rl_repo")

import numpy as np
import ml_dtypes

import concourse.bass as bass
import concourse.mybir as mybir
import concourse.tile as tile
from concourse import bacc
from concourse.bass_utils import run_bass_kernel_spmd

P = 128
B, N, C = 4, 2048, 1024
H, D = 16, 64
EMB = 1024
EPS = 1e-5
T = N          # tokens per batch on each core (k/v coverage)
TQ = N // 2    # query tokens per core
CH = C // P    # 8 channel chunks
O2 = 2 * C
NCORES = 8
TT = 512       # token tile in phase A1
NTT = TQ // TT   # 2 (h only for own query-half tokens)
NMT = T // 512   # 4  key-token tiles (512)
NMC = T // P     # 16 key-token chunks (128)
NNT = TQ // 512  # 2  query-token tiles (512)
LN8 = float(np.log(8.0))

F32 = mybir.dt.float32
BF16 = mybir.dt.bfloat16
MUL = mybir.AluOpType.mult
ADD = mybir.AluOpType.add
SUB = mybir.AluOpType.subtract
EXP = mybir.ActivationFunctionType.Exp
LN = mybir.ActivationFunctionType.Ln


def _ln_exp_set_id(nc):
    # act_info.json index of the set holding both Ln and Exp
    from concourse.hw_specs import get_activation_tables
    tables = get_activation_tables(nc.m.arch)
    for i, (name, funcs) in enumerate(tables.items()):
        if LN in funcs and EXP in funcs:
            return i
    raise RuntimeError("no ACT table set with both Ln and Exp")

_cached = {}


def build_kernel(debug=None):
    nc = bacc.Bacc(num_devices=NCORES)

    xT = nc.dram_tensor("xT", [C, T], BF16, kind="ExternalInput")
    embT = nc.dram_tensor("embT", [EMB, T], BF16, kind="ExternalInput")
    WembT = nc.dram_tensor("WembT", [EMB, O2], BF16, kind="ExternalInput")
    bemb = nc.dram_tensor("bemb", [P, O2 // P], F32, kind="ExternalInput")
    WprojT = nc.dram_tensor("WprojT", [C, 3 * C], BF16, kind="ExternalInput")
    WresT = nc.dram_tensor("WresT", [C, C], BF16, kind="ExternalInput")
    out = nc.dram_tensor("out", [TQ, C], F32, kind="ExternalOutput")

    RGROUPS = [[0, 1], [2, 3], [4, 5], [6, 7]]
    xT_r = xT.rearrange("(ch p) t -> p ch t", p=P)
    embT_r = embT.rearrange("(ch p) t -> p ch t", p=P)
    WembT_r = WembT.rearrange("(ch p) o -> p ch o", p=P)
    WprojT_r = WprojT.rearrange("(ch p) o -> p ch o", p=P)
    WresT_r = WresT.rearrange("(ch p) o -> p ch o", p=P)

    with tile.TileContext(nc) as tc:
        with (
            tc.tile_pool(name="const", bufs=1) as const,
            tc.tile_pool(name="main", bufs=1) as main,
            tc.tile_pool(name="dram", bufs=2, space="DRAM") as dram,
        ):
            # ---------------- constants ----------------
            eps_t = const.tile([P, 1], F32, name="eps_t")
            nc.vector.memset(eps_t[:], EPS)
            nln8_t = const.tile([P, 1], F32, name="nln8_t")
            nc.vector.memset(nln8_t[:], -LN8)
            ones_col = const.tile([P, 1], BF16, name="ones_col")
            nc.vector.memset(ones_col[:], 1.0)
            # preload the combined ln+exp ACT table set so the stats' Ln/Exp
            # never thrash tables against the softmax Exp stream
            nc.scalar.add_instruction(mybir.InstLoadActFuncSet(
                name=nc.get_next_instruction_name(),
                act_func_set_id=_ln_exp_set_id(nc)))
            # per-head partition-block reducers: head A -> out partition 0,
            # head B -> out partition 32 (32-aligned for engine slicing).
            bo8 = const.tile([P, 33], BF16, name="bo8")    # +1/8 blocks
            bo64 = const.tile([P, 33], BF16, name="bo64")  # +1/64 blocks
            for t_, v_ in ((bo8, 0.125), (bo64, 1.0 / 64)):
                nc.vector.memset(t_[:], 0.0)
                nc.vector.memset(t_[0:64, 0:1], v_)
                nc.vector.memset(t_[64:128, 32:33], v_)
            bemb_sb = const.tile([P, O2 // P], F32, name="bemb_sb")
            nc.sync.dma_start(bemb_sb[:], bemb[:])

            h_sb = main.tile([P, CH, TQ], BF16, name="h_sb")   # 16KB/part (own half)
            o_fm = main.tile([P, CH, TQ], BF16, name="o_fm")   # 16KB/part
            # out-proj weights: prefetch at kernel start so phase C never waits
            wres_sb = main.tile([P, CH, C], BF16, name="wres_sb")
            nc.sync.dma_start(wres_sb[:], WresT_r)

            # ============ Phase A1: se + LN(x) + FiLM -> h ============
            with (
                tc.tile_pool(name="wembp", bufs=1) as wembp,
                tc.tile_pool(name="a1s", bufs=2) as a1s,
                tc.tile_pool(name="a1r", bufs=2) as a1r,
                tc.tile_pool(name="ps_se", bufs=4, space="PSUM") as ps_se,
                tc.tile_pool(name="ps_st", bufs=2, space="PSUM") as ps_st,
            ):
                # x/e first (stats can start immediately), then wemb split
                # per output-chunk so the first se chains start early
                xe_t = []
                for tt in range(NTT):
                    tsl = slice(tt * TT, (tt + 1) * TT)
                    x_t = a1s.tile([P, CH, TT], BF16, name="x_t")
                    nc.sync.dma_start(x_t[:], xT_r[:, :, tsl])
                    e_t = a1s.tile([P, CH, TT], BF16, name="e_t")
                    nc.sync.dma_start(e_t[:], embT_r[:, :, tsl])
                    xe_t.append((x_t, e_t))
                wemb_sb = wembp.tile([P, CH, O2], BF16, name="wemb_sb")
                for ch in range(CH):
                    nc.sync.dma_start(wemb_sb[:, :, ch * P:(ch + 1) * P],
                                      WembT_r[:, :, ch * P:(ch + 1) * P])
                    nc.sync.dma_start(wemb_sb[:, :, C + ch * P:C + (ch + 1) * P],
                                      WembT_r[:, :, C + ch * P:C + (ch + 1) * P])

                for tt in range(NTT):
                    tsl = slice(tt * TT, (tt + 1) * TT)
                    x_t, e_t = xe_t[tt]

                    # LN stats over channels (partition reduction via matmul)
                    ps_s = ps_st.tile([1, TT], F32, name="ps_s", tag="ps_st")
                    ps_s2 = ps_st.tile([1, TT], F32, name="ps_s2", tag="ps_st")
                    for ch in range(CH):
                        x2c = a1r.tile([P, TT], BF16, name="x2c", tag="scr")
                        nc.vector.tensor_tensor(x2c[:], x_t[:, ch, :], x_t[:, ch, :], MUL)
                        nc.tensor.matmul(ps_s[:], ones_col[:], x_t[:, ch, :],
                                         start=(ch == 0), stop=(ch == CH - 1))
                        nc.tensor.matmul(ps_s2[:], ones_col[:], x2c[:],
                                         start=(ch == 0), stop=(ch == CH - 1))
                    mu = a1r.tile([1, TT], F32, name="mu")
                    m2 = a1r.tile([1, TT], F32, name="m2")
                    vr = a1r.tile([1, TT], F32, name="vr")
                    rows2 = a1r.tile([1, 2, TT], F32, name="rows2")  # [rstd, nmr]
                    nc.vector.tensor_scalar_mul(mu[:], ps_s[:], 1.0 / C)
                    nc.vector.tensor_tensor(m2[:], mu[:], mu[:], MUL)
                    nc.vector.scalar_tensor_tensor(vr[:], ps_s2[:], 1.0 / C, m2[:],
                                                   MUL, SUB)
                    # rstd = exp(-0.5 * ln(var + eps))
                    nc.scalar.activation(vr[:], vr[:], LN, bias=eps_t[0:1], scale=1.0)
                    nc.scalar.activation(rows2[:, 0, :], vr[:], EXP, bias=0.0, scale=-0.5)
                    nc.vector.scalar_tensor_tensor(rows2[:, 1, :], mu[:], -1.0,
                                                   rows2[:, 0, :], MUL, MUL)
                    bc2 = a1r.tile([P, 2, TT], F32, name="bc2")
                    nc.gpsimd.partition_broadcast(bc2[:], rows2[:])
                    rstd_bc = bc2[:, 0, :]
                    nmr_bc = bc2[:, 1, :]

                    for ch in range(CH):
                        ps_sc = ps_se.tile([P, TT], F32, name="ps_sc", tag="ps_se")
                        for ech in range(CH):
                            nc.tensor.matmul(ps_sc[:],
                                             wemb_sb[:, ech, ch * P:(ch + 1) * P],
                                             e_t[:, ech, :],
                                             start=(ech == 0), stop=(ech == CH - 1))
                        ps_sh = ps_se.tile([P, TT], F32, name="ps_sh", tag="ps_se")
                        for ech in range(CH):
                            nc.tensor.matmul(ps_sh[:],
                                             wemb_sb[:, ech, C + ch * P:C + (ch + 1) * P],
                                             e_t[:, ech, :],
                                             start=(ech == 0), stop=(ech == CH - 1))
                        xn = a1r.tile([P, TT], F32, name="xn", tag="scr2")
                        tm = a1r.tile([P, TT], F32, name="tm", tag="scr3")
                        nc.vector.tensor_tensor(xn[:], x_t[:, ch, :], rstd_bc, MUL)
                        nc.vector.tensor_tensor(xn[:], xn[:], nmr_bc, ADD)
                        # (sc + bemb_sc) * xn   (bemb has the FiLM +1 folded in)
                        nc.vector.scalar_tensor_tensor(tm[:], ps_sc[:],
                                                       bemb_sb[:, ch:ch + 1], xn[:],
                                                       ADD, MUL)
                        nc.vector.scalar_tensor_tensor(h_sb[:, ch, tsl], ps_sh[:],
                                                       bemb_sb[:, CH + ch:CH + ch + 1],
                                                       tm[:], ADD, ADD)

            if debug == "h":
                nc.gpsimd.dma_start(out.rearrange("(ch p) t -> p ch t", p=P),
                                    h_sb[:, :, 0:TQ])
            if debug != "h":
                # ============ Phase B: per-head-pair qkv + attention ============
                with (
                    tc.tile_pool(name="bw", bufs=2) as bw,
                    tc.tile_pool(name="batt", bufs=2) as batt,
                    tc.tile_pool(name="bk", bufs=2) as bk,
                    tc.tile_pool(name="bp", bufs=4) as bp,
                    tc.tile_pool(name="bsm", bufs=2) as bsm,
                    tc.tile_pool(name="ps_mm", bufs=2, space="PSUM") as ps_mm,
                    tc.tile_pool(name="ps_sc", bufs=2, space="PSUM") as ps_sc_p,
                    tc.tile_pool(name="ps_ot", bufs=2, space="PSUM") as ps_ot,
                ):
                    def produce_v(hq):
                        wv_sb = bw.tile([P, CH, 256], BF16, name="wv_sb")
                        nc.sync.dma_start(wv_sb[:], WprojT_r[:, :, 2 * C + hq * 256:2 * C + (hq + 1) * 256])
                        v_sb = batt.tile([P, NMC, 4, 72], BF16, name="v_sb")
                        nc.vector.memset(v_sb[:, :, :, 64:65], 1.0)
                        # own-half v -> DRAM -> pair AllGather -> all 16 chunks
                        vj_d = dram.tile([TQ, 256], BF16, name="vj_d")
                        vg_d = dram.tile([2, TQ, 256], BF16, name="vg_d")
                        for mtk in range(NMC // 2):
                            ps_v = ps_mm.tile([P, 4, 64], F32, name="ps_v", tag="ps_mm")
                            for ch in range(CH):
                                nc.tensor.matmul(ps_v[:], h_sb[:, ch, mtk * P:(mtk + 1) * P],
                                                 wv_sb[:, ch, :], start=(ch == 0), stop=(ch == CH - 1))
                            vtmp = bsm.tile([P, 256], BF16, name="vtmp")
                            nc.vector.tensor_copy(vtmp[:], ps_v[:])
                            nc.sync.dma_start(vj_d[mtk * P:(mtk + 1) * P, :], vtmp[:])
                        nc.gpsimd.collective_compute(
                            "AllGather", mybir.AluOpType.bypass,
                            replica_groups=RGROUPS,
                            ins=[vj_d[:]], outs=[vg_d[:]])
                        for r in range(2):
                            for i in range(NMC // 2):
                                nc.sync.dma_start(
                                    v_sb[:, r * 8 + i, :, 0:64],
                                    vg_d[r, i * P:(i + 1) * P, :].rearrange(
                                        "p (hh d) -> p hh d", hh=4))
                        return v_sb

                    def produce_kc(hp):
                            wqk_sb = bw.tile([P, CH, 256], BF16, name="wqk_sb")
                            nc.sync.dma_start(wqk_sb[:, :, 0:128], WprojT_r[:, :, hp * P:(hp + 1) * P])
                            nc.sync.dma_start(wqk_sb[:, :, 128:256],
                                              WprojT_r[:, :, C + hp * P:C + (hp + 1) * P])

                            # ---- k/q projection + row-space stats (own half) ----
                            # scl33 free layout: [2 (rk|mu), 4 (2 k-mts + 2 q-nts), 512]
                            # rows: head A at partition 0, head B at partition 32
                            NMTO = NMT // 2  # own-half k tiles
                            NCK = NMTO + NNT  # 4 chunks per hp
                            k2sb = bk.tile([P, NMTO, 512], BF16, name="k2sb")
                            q2sb = bk.tile([P, NNT, 512], BF16, name="q2sb")
                            scl33 = bk.tile([33, 2, NCK, 512], BF16, name="scl33")
                            v33 = bk.tile([33, NCK, 512], F32, name="v33")
                            for ck in range(NCK):
                                is_k = ck < NMTO
                                csl = slice(ck * 512, (ck + 1) * 512) if is_k else \
                                    slice((ck - NMTO) * 512, (ck - NMTO + 1) * 512)
                                wsl = slice(128, 256) if is_k else slice(0, 128)
                                dst = k2sb[:, ck, :] if is_k else q2sb[:, ck - NMTO, :]
                                ps_k = ps_mm.tile([P, 512], F32, name="ps_k", tag="ps_mm")
                                for ch in range(CH):
                                    nc.tensor.matmul(ps_k[:], wqk_sb[:, ch, wsl],
                                                     h_sb[:, ch, csl],
                                                     start=(ch == 0), stop=(ch == CH - 1))
                                nc.vector.tensor_copy(dst, ps_k[:])
                                ksq = bsm.tile([P, 512], BF16, name="ksq")
                                nc.vector.tensor_tensor(ksq[:], dst, dst, MUL)
                                ps_kr = ps_mm.tile([33, 512], F32, name="ps_kr", tag="ps_mm")
                                nc.tensor.matmul(ps_kr[:], bo8[:], dst, start=True, stop=True)
                                ps_kr2 = ps_mm.tile([33, 512], F32, name="ps_kr2", tag="ps_mm")
                                nc.tensor.matmul(ps_kr2[:], bo64[:], ksq[:], start=True, stop=True)
                                # mu = (8 mu)/8 ; var = E[k^2] - mu^2
                                nc.vector.tensor_scalar_mul(scl33[:, 1, ck, :], ps_kr[:], 0.125)
                                m2r = bsm.tile([33, 512], F32, name="m2r")
                                nc.vector.tensor_tensor(m2r[:], scl33[:, 1, ck, :],
                                                        scl33[:, 1, ck, :], MUL)
                                nc.vector.tensor_tensor(v33[:, ck, :], ps_kr2[:], m2r[:], SUB)
                            # batched rsqrt: rk = exp(-0.5 ln(var+eps))
                            nc.scalar.activation(v33[:], v33[:], LN, bias=eps_t[0:33], scale=1.0)
                            nc.scalar.activation(scl33[:, 0, :, :], v33[:], EXP, bias=0.0, scale=-0.5)

                            # kc/qc = (kq - mu) * rk, stacked [128, *] bf16
                            kco = bk.tile([P, NMTO, 512], BF16, name="kco")
                            kc = bk.tile([P, NMT, 512], BF16, name="kc")
                            qc = bk.tile([P, NNT, 512], BF16, name="qc")
                            for ck in range(NCK):
                                is_k = ck < NMTO
                                src = kco[:, ck, :] if is_k else q2sb[:, ck - NMTO, :]
                                src = k2sb[:, ck, :] if is_k else q2sb[:, ck - NMTO, :]
                                dst = kco[:, ck, :] if is_k else qc[:, ck - NMTO, :]
                                sclB0 = bsm.tile([1, 2, 512], BF16, name="sclB0")
                                nc.sync.dma_start(sclB0[:], scl33[32:33, :, ck, :])
                                bc = bsm.tile([P, 2, 512], BF16, name="bc")
                                # partition_broadcast only writes from a tile's
                                # partition 0: fill all 128 with head B, then
                                # overwrite the top half with head A.
                                nc.gpsimd.partition_broadcast(bc[:], sclB0[:])
                                nc.gpsimd.partition_broadcast(bc[0:64, :, :],
                                                              scl33[0:1, :, ck, :])
                                kct = bsm.tile([P, 512], F32, name="kct")
                                nc.vector.tensor_tensor(kct[:], src, bc[:, 1, :], SUB)
                                nc.vector.tensor_tensor(dst, kct[:], bc[:, 0, :], MUL)
                            # gather centered k across the core pair
                            kcj_d = dram.tile([P, TQ], BF16, name="kcj_d")
                            kcg_d = dram.tile([2, P, TQ], BF16, name="kcg_d")
                            nc.sync.dma_start(kcj_d[:], kco[:])
                            nc.gpsimd.collective_compute(
                                "AllGather", mybir.AluOpType.bypass,
                                replica_groups=RGROUPS,
                                ins=[kcj_d[:]], outs=[kcg_d[:]])
                            for r in range(2):
                                nc.sync.dma_start(kc[:, 2 * r:2 * r + 2, :],
                                                  kcg_d[r].rearrange("p (a b) -> p a b", a=NMTO))
                            return kc, qc

                    def attention(hp, kc, qc, v_sb):
                            if debug == "qa":
                                ofl = out.rearrange("(a b) t -> a (b t)", a=P)
                                nc.gpsimd.dma_start(ofl[:, 0:TQ], qc[:])
                                nc.gpsimd.dma_start(ofl[:, TQ:TQ + T], kc[:])
                                return

                            # ---- scores + exp + o per head pair ----
                            hA, hB = 2 * hp, 2 * hp + 1
                            viA = (hp % 2) * 2
                            viB = viA + 1
                            for nt in range(NNT):
                                nsl = slice(nt * 512, (nt + 1) * 512)
                                ps_oA = ps_ot.tile([65, 512], F32, name="ps_oA", tag="ps_ot")
                                ps_oB = ps_ot.tile([65, 512], F32, name="ps_oB", tag="ps_ot")
                                for mc in range(NMC):
                                    mt, off = mc // 4, (mc % 4) * P
                                    ps_s = ps_sc_p.tile([P, 2, 512], F32, name="ps_s", tag="ps_sc")
                                    nc.tensor.matmul(ps_s[:, 0, :],
                                                     kc[0:64, mt, off:off + P],
                                                     qc[0:64, nt, :], start=True, stop=True)
                                    nc.tensor.matmul(ps_s[:, 1, :],
                                                     kc[64:128, mt, off:off + P],
                                                     qc[64:128, nt, :], start=True, stop=True)
                                    p_t = bp.tile([P, 2, 512], BF16, name="p_t")
                                    nc.scalar.activation(p_t[:], ps_s[:], EXP, bias=0.0, scale=0.125)
                                    nc.tensor.matmul(ps_oA[:], v_sb[:, mc, viA, 0:65],
                                                     p_t[:, 0, :],
                                                     start=(mc == 0), stop=(mc == NMC - 1))
                                    nc.tensor.matmul(ps_oB[:], v_sb[:, mc, viB, 0:65],
                                                     p_t[:, 1, :],
                                                     start=(mc == 0), stop=(mc == NMC - 1))
                                # softmax division fused into the evacuation:
                                # den row hops to partition 0 (aligned -64),
                                # reciprocal, gpsimd-broadcast, then
                                # o_fm = ps_o[0:64] * rec_bc in one DVE op.
                                for ps_oX, hX in ((ps_oA, hA), (ps_oB, hB)):
                                    den_row = bsm.tile([1, 512], F32, name="den_row")
                                    nc.vector.tensor_copy(den_row[:], ps_oX[64:65, :])
                                    nc.vector.reciprocal_approx_fast(den_row[:], den_row[:])
                                    dbc = bsm.tile([64, 512], F32, name="dbc")
                                    nc.gpsimd.partition_broadcast(dbc[:], den_row[:])
                                    nc.vector.tensor_tensor(
                                        o_fm[(hX % 2) * 64:(hX % 2) * 64 + 64, hX // 2, nsl],
                                        ps_oX[0:64, :], dbc[:], MUL)

                    # one-ahead software pipeline: kc/v gathers for step hp+1
                    # fly while step hp's attention keeps the PE busy
                    v_tiles = {0: produce_v(0)}
                    kq = {0: produce_kc(0)}
                    for hp in range(H // 2):
                        if hp + 1 < H // 2:
                            if (hp + 1) % 2 == 0:
                                v_tiles[(hp + 1) // 2] = produce_v((hp + 1) // 2)
                            kq[hp + 1] = produce_kc(hp + 1)
                        kc_, qc_ = kq.pop(hp)
                        attention(hp, kc_, qc_, v_tiles[hp // 2])

                if debug == "b1":
                    nc.gpsimd.dma_start(out.rearrange("(ch p) t -> p ch t", p=P),
                                        o_fm[:, :, :])
                # ============ Phase C: out = o_fm.T @ (I + W_out).T ============
                if debug is None:
                  with (
                      tc.tile_pool(name="cw", bufs=2) as cw,
                      tc.tile_pool(name="ps_c", bufs=2, space="PSUM") as ps_c,
                  ):
                    for jt in range(C // 512):
                        jsl = slice(jt * 512, (jt + 1) * 512)
                        for ns in range(TQ // P):
                            ps_f = ps_c.tile([P, 512], F32, name="ps_f", tag="ps_c")
                            for cg in range(CH):
                                nc.tensor.matmul(ps_f[:], o_fm[:, cg, ns * P:(ns + 1) * P],
                                                 wres_sb[:, cg, jsl],
                                                 start=(cg == 0), stop=(cg == CH - 1))
                            f_sb = cw.tile([P, 512], F32, name="f_sb")
                            nc.vector.tensor_copy(f_sb[:], ps_f[:])
                            nc.sync.dma_start(out[ns * P:(ns + 1) * P, jt * 512:(jt + 1) * 512],
                                              f_sb[:])

    nc.finalize()
    return nc


def _prep_host(x, emb, W_emb, b_emb, W_proj, W_out):
    bf16 = ml_dtypes.bfloat16
    W_embT = np.ascontiguousarray(W_emb.T.astype(bf16))
    W_projT = np.ascontiguousarray(W_proj.T.astype(bf16))
    W_resT = np.ascontiguousarray((np.eye(C, dtype=np.float32) + W_out).T.astype(bf16))
    bemb2 = b_emb.astype(np.float32).copy()
    bemb2[:C] += 1.0                       # fold the FiLM "+1" into the bias
    bemb_col = np.ascontiguousarray(bemb2.reshape(O2 // P, P).T)

    in_maps = []
    for c in range(NCORES):
        b, j = c // 2, c % 2
        perm = np.concatenate([np.arange(j * TQ, (j + 1) * TQ),
                               np.arange((1 - j) * TQ, (2 - j) * TQ)])
        in_maps.append({
            "xT": np.ascontiguousarray(x[b][perm].T.astype(bf16)),
            "embT": np.ascontiguousarray(emb[b][perm].T.astype(bf16)),
            "WembT": W_embT, "bemb": bemb_col,
            "WprojT": W_projT, "WresT": W_resT,
        })
    return in_maps


def kernel(x, emb, W_emb, b_emb, W_proj, W_out, _trace=False, _tmpdir=None, _debug=None):
    x = np.asarray(x); emb = np.asarray(emb)
    W_emb = np.asarray(W_emb); b_emb = np.asarray(b_emb)
    W_proj = np.asarray(W_proj); W_out = np.asarray(W_out)

    key = ("nc", _debug)
    if key not in _cached:
        _cached[key] = build_kernel(debug=_debug)
    nc = _cached[key]

    in_maps = _prep_host(x, emb, W_emb, b_emb, W_proj, W_out)
    res = run_bass_kernel_spmd(nc, in_maps, core_ids=list(range(NCORES)), trace=_trace,
                               tmpdir=_tmpdir)
    _cached["last_result"] = res

    outp = np.empty((B, N, C), dtype=np.float32)
    for c in range(NCORES):
        b, j = c // 2, c % 2
        outp[b, j * TQ:(j + 1) * TQ, :] = res.results[c]["out"]
    return outp


# revision 36
# speedup vs baseline: 1.2346x; 1.2346x over previous
"""AttentionBlock Trainium2 kernel (B=4, N=2048, C=1024, H=16, D=64, EMB=1024).

    se = emb @ W_emb.T + b_emb;  scale, shift = split(se, 2, -1)
    h  = LN(x) * (1+scale) + shift
    q,k,v = split(h @ W_proj.T) -> (B,H,N,D);  q = LN(q); k = LN(k)  (over D)
    o  = softmax(q k^T / sqrt(D)) v  -> (B,N,C)
    out = o + o @ W_out.T

Sharding: 8 cores; core c -> batch b=c//2, query-half j=c%2. The host rolls
the token axis per core so its query tokens are always tokens 0:1024. Each
core computes the preamble (se/h/qkv/LN) for ONLY its 1024 tokens; centered
keys (kc) and values are then AllGathered across the core pair ([0,1],[2,3],
[4,5],[6,7]) in rank-major key order (attention is permutation-equivariant
over keys), and each core runs attention + out-proj for its 1024 query rows.

Dataflow is feature-major (channels on partitions) end to end; all matmul
operands are bf16 (PSUM accumulation stays fp32):
  - LayerNorm over channels == partition reduction -> ones-column matmuls;
    per-token (free-dim) row stats broadcast across partitions with gpsimd
    partition_broadcast (no DRAM bounce). partition_broadcast only reads a
    tile's partition 0 and only writes base-0 dests, so head-B rows hop to
    partition 0 via a tiny SBUF->SBUF DMA, and two-half tiles are built as
    full-tile broadcast (B) + top-half overwrite (A).
  - q/k LN is folded in *before* the score matmul: kc = (k - mu_k)*rk and
    qc = (q - mu_q)*rq in stacked [128, T] layouts (head pair on partition
    halves), so scores are two concurrent row-tiled K=64 matmuls (tile rows
    0-63 / 64-127) and the softmax exp needs no per-partition scale: one
    ACTIVATE covers both heads' PSUM banks with the 1/sqrt(D) folded into
    its affine scale (0.125).
  - Per-head per-token mean/E[x^2] come from [P,33]-column reducer matmuls
    (head A -> partition 0, head B -> partition 32, both 32-aligned for
    engine slicing); transforms run on the full [33,*] rows (DVE cost is
    per-partition-free-size, the garbage rows are free).
  - rstd/rk/rq = exp(-0.5*ln(var+eps)); a manually emitted InstLoadActFuncSet
    of the natural_log_exp_and_others set keeps the scalar engine on ONE
    table set for the whole kernel (no thrash against the softmax Exp).
  - Softmax denominators come free as a ones column appended to v (row 64 of
    the attnv accumulator); division is fused into the o evacuation via
    reciprocal_approx_fast + gpsimd broadcast.
  - The residual is folded into the output projection: W_res = (I+W_out).T,
    prefetched at kernel start so phase C never stalls.
"""

import sys

sys.path.insert(0, "/opt/trn_rl_repo")

import numpy as np
import ml_dtypes

import concourse.bass as bass
import concourse.mybir as mybir
import concourse.tile as tile
from concourse import bacc
from concourse.bass_utils import run_bass_kernel_spmd

P = 128
B, N, C = 4, 2048, 1024
H, D = 16, 64
EMB = 1024
EPS = 1e-5
T = N          # tokens per batch on each core (k/v coverage)
TQ = N // 2    # query tokens per core
CH = C // P    # 8 channel chunks
O2 = 2 * C
NCORES = 8
TT = 512       # token tile in phase A1
NTT = TQ // TT   # 2 (h only for own query-half tokens)
NMT = T // 512   # 4  key-token tiles (512)
NMC = T // P     # 16 key-token chunks (128)
NNT = TQ // 512  # 2  query-token tiles (512)
LN8 = float(np.log(8.0))

F32 = mybir.dt.float32
BF16 = mybir.dt.bfloat16
MUL = mybir.AluOpType.mult
ADD = mybir.AluOpType.add
SUB = mybir.AluOpType.subtract
EXP = mybir.ActivationFunctionType.Exp
LN = mybir.ActivationFunctionType.Ln


def _ln_exp_set_id(nc):
    # act_info.json index of the set holding both Ln and Exp
    from concourse.hw_specs import get_activation_tables
    tables = get_activation_tables(nc.m.arch)
    for i, (name, funcs) in enumerate(tables.items()):
        if LN in funcs and EXP in funcs:
            return i
    raise RuntimeError("no ACT table set with both Ln and Exp")

_cached = {}


def build_kernel(debug=None):
    nc = bacc.Bacc(num_devices=NCORES)

    xT = nc.dram_tensor("xT", [C, T], BF16, kind="ExternalInput")
    embT = nc.dram_tensor("embT", [EMB, T], BF16, kind="ExternalInput")
    WembT = nc.dram_tensor("WembT", [EMB, O2], BF16, kind="ExternalInput")
    bemb = nc.dram_tensor("bemb", [P, O2 // P], F32, kind="ExternalInput")
    WprojT = nc.dram_tensor("WprojT", [C, 3 * C], BF16, kind="ExternalInput")
    WresT = nc.dram_tensor("WresT", [C, C], BF16, kind="ExternalInput")
    out = nc.dram_tensor("out", [TQ, C], F32, kind="ExternalOutput")

    RGROUPS = [[0, 1], [2, 3], [4, 5], [6, 7]]
    xT_r = xT.rearrange("(ch p) t -> p ch t", p=P)
    embT_r = embT.rearrange("(ch p) t -> p ch t", p=P)
    WembT_r = WembT.rearrange("(ch p) o -> p ch o", p=P)
    WprojT_r = WprojT.rearrange("(ch p) o -> p ch o", p=P)
    WresT_r = WresT.rearrange("(ch p) o -> p ch o", p=P)

    with tile.TileContext(nc) as tc:
        with (
            tc.tile_pool(name="const", bufs=1) as const,
            tc.tile_pool(name="main", bufs=1) as main,
            tc.tile_pool(name="dram", bufs=2, space="DRAM") as dram,
        ):
            # ---------------- constants ----------------
            eps_t = const.tile([P, 1], F32, name="eps_t")
            nc.vector.memset(eps_t[:], EPS)
            nln8_t = const.tile([P, 1], F32, name="nln8_t")
            nc.vector.memset(nln8_t[:], -LN8)
            ones_col = const.tile([P, 1], BF16, name="ones_col")
            nc.vector.memset(ones_col[:], 1.0)
            # preload the combined ln+exp ACT table set so the stats' Ln/Exp
            # never thrash tables against the softmax Exp stream
            nc.scalar.add_instruction(mybir.InstLoadActFuncSet(
                name=nc.get_next_instruction_name(),
                act_func_set_id=_ln_exp_set_id(nc)))
            # per-head partition-block reducers: head A -> out partition 0,
            # head B -> out partition 32 (32-aligned for engine slicing).
            bo8 = const.tile([P, 33], BF16, name="bo8")    # +1/8 blocks
            bo64 = const.tile([P, 33], BF16, name="bo64")  # +1/64 blocks
            for t_, v_ in ((bo8, 0.125), (bo64, 1.0 / 64)):
                nc.vector.memset(t_[:], 0.0)
                nc.vector.memset(t_[0:64, 0:1], v_)
                nc.vector.memset(t_[64:128, 32:33], v_)
            bemb_sb = const.tile([P, O2 // P], F32, name="bemb_sb")
            nc.sync.dma_start(bemb_sb[:], bemb[:])

            h_sb = main.tile([P, CH, TQ], BF16, name="h_sb")   # 16KB/part (own half)
            o_fm = main.tile([P, CH, TQ], BF16, name="o_fm")   # 16KB/part
            # out-proj weights: prefetch at kernel start so phase C never waits
            wres_sb = main.tile([P, CH, C], BF16, name="wres_sb")
            nc.sync.dma_start(wres_sb[:], WresT_r)

            # ============ Phase A1: se + LN(x) + FiLM -> h ============
            with (
                tc.tile_pool(name="wembp", bufs=1) as wembp,
                tc.tile_pool(name="a1s", bufs=2) as a1s,
                tc.tile_pool(name="a1r", bufs=2) as a1r,
                tc.tile_pool(name="ps_se", bufs=4, space="PSUM") as ps_se,
                tc.tile_pool(name="ps_st", bufs=2, space="PSUM") as ps_st,
            ):
                # x/e first (stats can start immediately), then wemb split
                # per output-chunk so the first se chains start early
                xe_t = []
                for tt in range(NTT):
                    tsl = slice(tt * TT, (tt + 1) * TT)
                    x_t = a1s.tile([P, CH, TT], BF16, name="x_t")
                    nc.sync.dma_start(x_t[:], xT_r[:, :, tsl])
                    e_t = a1s.tile([P, CH, TT], BF16, name="e_t")
                    nc.sync.dma_start(e_t[:], embT_r[:, :, tsl])
                    xe_t.append((x_t, e_t))
                wemb_sb = wembp.tile([P, CH, O2], BF16, name="wemb_sb")
                for ch in range(CH):
                    nc.sync.dma_start(wemb_sb[:, :, ch * P:(ch + 1) * P],
                                      WembT_r[:, :, ch * P:(ch + 1) * P])
                    nc.sync.dma_start(wemb_sb[:, :, C + ch * P:C + (ch + 1) * P],
                                      WembT_r[:, :, C + ch * P:C + (ch + 1) * P])

                for tt in range(NTT):
                    tsl = slice(tt * TT, (tt + 1) * TT)
                    x_t, e_t = xe_t[tt]

                    # LN stats over channels (partition reduction via matmul)
                    ps_s = ps_st.tile([1, TT], F32, name="ps_s", tag="ps_st")
                    ps_s2 = ps_st.tile([1, TT], F32, name="ps_s2", tag="ps_st")
                    for ch in range(CH):
                        x2c = a1r.tile([P, TT], BF16, name="x2c", tag="scr")
                        nc.vector.tensor_tensor(x2c[:], x_t[:, ch, :], x_t[:, ch, :], MUL)
                        nc.tensor.matmul(ps_s[:], ones_col[:], x_t[:, ch, :],
                                         start=(ch == 0), stop=(ch == CH - 1))
                        nc.tensor.matmul(ps_s2[:], ones_col[:], x2c[:],
                                         start=(ch == 0), stop=(ch == CH - 1))
                    mu = a1r.tile([1, TT], F32, name="mu")
                    m2 = a1r.tile([1, TT], F32, name="m2")
                    vr = a1r.tile([1, TT], F32, name="vr")
                    rows2 = a1r.tile([1, 2, TT], F32, name="rows2")  # [rstd, nmr]
                    nc.vector.tensor_scalar_mul(mu[:], ps_s[:], 1.0 / C)
                    nc.vector.tensor_tensor(m2[:], mu[:], mu[:], MUL)
                    nc.vector.scalar_tensor_tensor(vr[:], ps_s2[:], 1.0 / C, m2[:],
                                                   MUL, SUB)
                    # rstd = exp(-0.5 * ln(var + eps))
                    nc.scalar.activation(vr[:], vr[:], LN, bias=eps_t[0:1], scale=1.0)
                    nc.scalar.activation(rows2[:, 0, :], vr[:], EXP, bias=0.0, scale=-0.5)
                    nc.vector.scalar_tensor_tensor(rows2[:, 1, :], mu[:], -1.0,
                                                   rows2[:, 0, :], MUL, MUL)
                    bc2 = a1r.tile([P, 2, TT], F32, name="bc2")
                    nc.gpsimd.partition_broadcast(bc2[:], rows2[:])
                    rstd_bc = bc2[:, 0, :]
                    nmr_bc = bc2[:, 1, :]

                    for ch in range(CH):
                        ps_sc = ps_se.tile([P, TT], F32, name="ps_sc", tag="ps_se")
                        for ech in range(CH):
                            nc.tensor.matmul(ps_sc[:],
                                             wemb_sb[:, ech, ch * P:(ch + 1) * P],
                                             e_t[:, ech, :],
                                             start=(ech == 0), stop=(ech == CH - 1))
                        ps_sh = ps_se.tile([P, TT], F32, name="ps_sh", tag="ps_se")
                        for ech in range(CH):
                            nc.tensor.matmul(ps_sh[:],
                                             wemb_sb[:, ech, C + ch * P:C + (ch + 1) * P],
                                             e_t[:, ech, :],
                                             start=(ech == 0), stop=(ech == CH - 1))
                        xn = a1r.tile([P, TT], F32, name="xn", tag="scr2")
                        tm = a1r.tile([P, TT], F32, name="tm", tag="scr3")
                        nc.vector.tensor_tensor(xn[:], x_t[:, ch, :], rstd_bc, MUL)
                        nc.vector.tensor_tensor(xn[:], xn[:], nmr_bc, ADD)
                        # (sc + bemb_sc) * xn   (bemb has the FiLM +1 folded in)
                        nc.vector.scalar_tensor_tensor(tm[:], ps_sc[:],
                                                       bemb_sb[:, ch:ch + 1], xn[:],
                                                       ADD, MUL)
                        nc.vector.scalar_tensor_tensor(h_sb[:, ch, tsl], ps_sh[:],
                                                       bemb_sb[:, CH + ch:CH + ch + 1],
                                                       tm[:], ADD, ADD)

            if debug == "h":
                nc.gpsimd.dma_start(out.rearrange("(ch p) t -> p ch t", p=P),
                                    h_sb[:, :, 0:TQ])
            if debug != "h":
                # ============ Phase B: per-head-pair qkv + attention ============
                with (
                    tc.tile_pool(name="bw", bufs=2) as bw,
                    tc.tile_pool(name="batt", bufs=1) as batt,
                    tc.tile_pool(name="bk", bufs=2) as bk,
                    tc.tile_pool(name="bp", bufs=4) as bp,
                    tc.tile_pool(name="bsm", bufs=2) as bsm,
                    tc.tile_pool(name="ps_mm", bufs=2, space="PSUM") as ps_mm,
                    tc.tile_pool(name="ps_sc", bufs=2, space="PSUM") as ps_sc_p,
                    tc.tile_pool(name="ps_ot", bufs=2, space="PSUM") as ps_ot,
                ):
                    for hq in range(4):  # head quads
                        wv_sb = bw.tile([P, CH, 256], BF16, name="wv_sb")
                        nc.sync.dma_start(wv_sb[:], WprojT_r[:, :, 2 * C + hq * 256:2 * C + (hq + 1) * 256])
                        v_sb = batt.tile([P, NMC, 4, 72], BF16, name="v_sb")
                        nc.vector.memset(v_sb[:, :, :, 64:65], 1.0)
                        # own-half v -> DRAM -> pair AllGather -> all 16 chunks
                        vj_d = dram.tile([TQ, 256], BF16, name="vj_d")
                        vg_d = dram.tile([2, TQ, 256], BF16, name="vg_d")
                        for mtk in range(NMC // 2):
                            ps_v = ps_mm.tile([P, 4, 64], F32, name="ps_v", tag="ps_mm")
                            for ch in range(CH):
                                nc.tensor.matmul(ps_v[:], h_sb[:, ch, mtk * P:(mtk + 1) * P],
                                                 wv_sb[:, ch, :], start=(ch == 0), stop=(ch == CH - 1))
                            vtmp = bsm.tile([P, 256], BF16, name="vtmp")
                            nc.vector.tensor_copy(vtmp[:], ps_v[:])
                            nc.sync.dma_start(vj_d[mtk * P:(mtk + 1) * P, :], vtmp[:])
                        nc.gpsimd.collective_compute(
                            "AllGather", mybir.AluOpType.bypass,
                            replica_groups=RGROUPS,
                            ins=[vj_d[:]], outs=[vg_d[:]])
                        for r in range(2):
                            for i in range(NMC // 2):
                                nc.sync.dma_start(
                                    v_sb[:, r * 8 + i, :, 0:64],
                                    vg_d[r, i * P:(i + 1) * P, :].rearrange(
                                        "p (hh d) -> p hh d", hh=4))

                        for hp in (2 * hq, 2 * hq + 1):
                            wqk_sb = bw.tile([P, CH, 256], BF16, name="wqk_sb")
                            nc.sync.dma_start(wqk_sb[:, :, 0:128], WprojT_r[:, :, hp * P:(hp + 1) * P])
                            nc.sync.dma_start(wqk_sb[:, :, 128:256],
                                              WprojT_r[:, :, C + hp * P:C + (hp + 1) * P])

                            # ---- k/q projection + row-space stats (own half) ----
                            # scl33 free layout: [2 (rk|mu), 4 (2 k-mts + 2 q-nts), 512]
                            # rows: head A at partition 0, head B at partition 32
                            NMTO = NMT // 2  # own-half k tiles
                            NCK = NMTO + NNT  # 4 chunks per hp
                            k2sb = bk.tile([P, NMTO, 512], BF16, name="k2sb")
                            q2sb = bk.tile([P, NNT, 512], BF16, name="q2sb")
                            scl33 = bk.tile([33, 2, NCK, 512], BF16, name="scl33")
                            v33 = bk.tile([33, NCK, 512], F32, name="v33")
                            for ck in range(NCK):
                                is_k = ck < NMTO
                                csl = slice(ck * 512, (ck + 1) * 512) if is_k else \
                                    slice((ck - NMTO) * 512, (ck - NMTO + 1) * 512)
                                wsl = slice(128, 256) if is_k else slice(0, 128)
                                dst = k2sb[:, ck, :] if is_k else q2sb[:, ck - NMTO, :]
                                ps_k = ps_mm.tile([P, 512], F32, name="ps_k", tag="ps_mm")
                                for ch in range(CH):
                                    nc.tensor.matmul(ps_k[:], wqk_sb[:, ch, wsl],
                                                     h_sb[:, ch, csl],
                                                     start=(ch == 0), stop=(ch == CH - 1))
                                nc.vector.tensor_copy(dst, ps_k[:])
                                ksq = bsm.tile([P, 512], BF16, name="ksq")
                                nc.vector.tensor_tensor(ksq[:], dst, dst, MUL)
                                ps_kr = ps_mm.tile([33, 512], F32, name="ps_kr", tag="ps_mm")
                                nc.tensor.matmul(ps_kr[:], bo8[:], dst, start=True, stop=True)
                                ps_kr2 = ps_mm.tile([33, 512], F32, name="ps_kr2", tag="ps_mm")
                                nc.tensor.matmul(ps_kr2[:], bo64[:], ksq[:], start=True, stop=True)
                                # mu = (8 mu)/8 ; var = E[k^2] - mu^2
                                nc.vector.tensor_scalar_mul(scl33[:, 1, ck, :], ps_kr[:], 0.125)
                                m2r = bsm.tile([33, 512], F32, name="m2r")
                                nc.vector.tensor_tensor(m2r[:], scl33[:, 1, ck, :],
                                                        scl33[:, 1, ck, :], MUL)
                                nc.vector.tensor_tensor(v33[:, ck, :], ps_kr2[:], m2r[:], SUB)
                            # batched rsqrt: rk = exp(-0.5 ln(var+eps))
                            nc.scalar.activation(v33[:], v33[:], LN, bias=eps_t[0:33], scale=1.0)
                            nc.scalar.activation(scl33[:, 0, :, :], v33[:], EXP, bias=0.0, scale=-0.5)

                            # kc/qc = (kq - mu) * rk, stacked [128, *] bf16
                            kco = bk.tile([P, NMTO, 512], BF16, name="kco")
                            kc = bk.tile([P, NMT, 512], BF16, name="kc")
                            qc = bk.tile([P, NNT, 512], BF16, name="qc")
                            for ck in range(NCK):
                                is_k = ck < NMTO
                                src = kco[:, ck, :] if is_k else q2sb[:, ck - NMTO, :]
                                src = k2sb[:, ck, :] if is_k else q2sb[:, ck - NMTO, :]
                                dst = kco[:, ck, :] if is_k else qc[:, ck - NMTO, :]
                                sclB0 = bsm.tile([1, 2, 512], BF16, name="sclB0")
                                nc.sync.dma_start(sclB0[:], scl33[32:33, :, ck, :])
                                bc = bsm.tile([P, 2, 512], BF16, name="bc")
                                # partition_broadcast only writes from a tile's
                                # partition 0: fill all 128 with head B, then
                                # overwrite the top half with head A.
                                nc.gpsimd.partition_broadcast(bc[:], sclB0[:])
                                nc.gpsimd.partition_broadcast(bc[0:64, :, :],
                                                              scl33[0:1, :, ck, :])
                                kct = bsm.tile([P, 512], F32, name="kct")
                                nc.vector.tensor_tensor(kct[:], src, bc[:, 1, :], SUB)
                                nc.vector.tensor_tensor(dst, kct[:], bc[:, 0, :], MUL)
                            # gather centered k across the core pair
                            kcj_d = dram.tile([P, TQ], BF16, name="kcj_d")
                            kcg_d = dram.tile([2, P, TQ], BF16, name="kcg_d")
                            nc.sync.dma_start(kcj_d[:], kco[:])
                            nc.gpsimd.collective_compute(
                                "AllGather", mybir.AluOpType.bypass,
                                replica_groups=RGROUPS,
                                ins=[kcj_d[:]], outs=[kcg_d[:]])
                            for r in range(2):
                                nc.sync.dma_start(kc[:, 2 * r:2 * r + 2, :],
                                                  kcg_d[r].rearrange("p (a b) -> p a b", a=NMTO))

                            if debug == "qa":
                                ofl = out.rearrange("(a b) t -> a (b t)", a=P)
                                nc.gpsimd.dma_start(ofl[:, 0:TQ], qc[:])
                                nc.gpsimd.dma_start(ofl[:, TQ:TQ + T], kc[:])
                                continue

                            # ---- scores + exp + o per head pair ----
                            hA, hB = 2 * hp, 2 * hp + 1
                            viA = (hp % 2) * 2
                            viB = viA + 1
                            for nt in range(NNT):
                                nsl = slice(nt * 512, (nt + 1) * 512)
                                ps_oA = ps_ot.tile([65, 512], F32, name="ps_oA", tag="ps_ot")
                                ps_oB = ps_ot.tile([65, 512], F32, name="ps_oB", tag="ps_ot")
                                for mc in range(NMC):
                                    mt, off = mc // 4, (mc % 4) * P
                                    ps_s = ps_sc_p.tile([P, 2, 512], F32, name="ps_s", tag="ps_sc")
                                    nc.tensor.matmul(ps_s[:, 0, :],
                                                     kc[0:64, mt, off:off + P],
                                                     qc[0:64, nt, :], start=True, stop=True)
                                    nc.tensor.matmul(ps_s[:, 1, :],
                                                     kc[64:128, mt, off:off + P],
                                                     qc[64:128, nt, :], start=True, stop=True)
                                    p_t = bp.tile([P, 2, 512], BF16, name="p_t")
                                    nc.scalar.activation(p_t[:], ps_s[:], EXP, bias=0.0, scale=0.125)
                                    nc.tensor.matmul(ps_oA[:], v_sb[:, mc, viA, 0:65],
                                                     p_t[:, 0, :],
                                                     start=(mc == 0), stop=(mc == NMC - 1))
                                    nc.tensor.matmul(ps_oB[:], v_sb[:, mc, viB, 0:65],
                                                     p_t[:, 1, :],
                                                     start=(mc == 0), stop=(mc == NMC - 1))
                                # softmax division fused into the evacuation:
                                # den row hops to partition 0 (aligned -64),
                                # reciprocal, gpsimd-broadcast, then
                                # o_fm = ps_o[0:64] * rec_bc in one DVE op.
                                for ps_oX, hX in ((ps_oA, hA), (ps_oB, hB)):
                                    den_row = bsm.tile([1, 512], F32, name="den_row")
                                    nc.vector.tensor_copy(den_row[:], ps_oX[64:65, :])
                                    nc.vector.reciprocal_approx_fast(den_row[:], den_row[:])
                                    dbc = bsm.tile([64, 512], F32, name="dbc")
                                    nc.gpsimd.partition_broadcast(dbc[:], den_row[:])
                                    nc.vector.tensor_tensor(
                                        o_fm[(hX % 2) * 64:(hX % 2) * 64 + 64, hX // 2, nsl],
                                        ps_oX[0:64, :], dbc[:], MUL)

                if debug == "b1":
                    nc.gpsimd.dma_start(out.rearrange("(ch p) t -> p ch t", p=P),
                                        o_fm[:, :, :])
                # ============ Phase C: out = o_fm.T @ (I + W_out).T ============
                if debug is None:
                  with (
                      tc.tile_pool(name="cw", bufs=2) as cw,
                      tc.tile_pool(name="ps_c", bufs=2, space="PSUM") as ps_c,
                  ):
                    for jt in range(C // 512):
                        jsl = slice(jt * 512, (jt + 1) * 512)
                        for ns in range(TQ // P):
                            ps_f = ps_c.tile([P, 512], F32, name="ps_f", tag="ps_c")
                            for cg in range(CH):
                                nc.tensor.matmul(ps_f[:], o_fm[:, cg, ns * P:(ns + 1) * P],
                                                 wres_sb[:, cg, jsl],
                                                 start=(cg == 0), stop=(cg == CH - 1))
                            f_sb = cw.tile([P, 512], F32, name="f_sb")
                            nc.vector.tensor_copy(f_sb[:], ps_f[:])
                            nc.sync.dma_start(out[ns * P:(ns + 1) * P, jt * 512:(jt + 1) * 512],
                                              f_sb[:])

    nc.finalize()
    return nc


def _prep_host(x, emb, W_emb, b_emb, W_proj, W_out):
    bf16 = ml_dtypes.bfloat16
    W_embT = np.ascontiguousarray(W_emb.T.astype(bf16))
    W_projT = np.ascontiguousarray(W_proj.T.astype(bf16))
    W_resT = np.ascontiguousarray((np.eye(C, dtype=np.float32) + W_out).T.astype(bf16))
    bemb2 = b_emb.astype(np.float32).copy()
    bemb2[:C] += 1.0                       # fold the FiLM "+1" into the bias
    bemb_col = np.ascontiguousarray(bemb2.reshape(O2 // P, P).T)

    in_maps = []
    for c in range(NCORES):
        b, j = c // 2, c % 2
        perm = np.concatenate([np.arange(j * TQ, (j + 1) * TQ),
                               np.arange((1 - j) * TQ, (2 - j) * TQ)])
        in_maps.append({
            "xT": np.ascontiguousarray(x[b][perm].T.astype(bf16)),
            "embT": np.ascontiguousarray(emb[b][perm].T.astype(bf16)),
            "WembT": W_embT, "bemb": bemb_col,
            "WprojT": W_projT, "WresT": W_resT,
        })
    return in_maps


def kernel(x, emb, W_emb, b_emb, W_proj, W_out, _trace=False, _tmpdir=None, _debug=None):
    x = np.asarray(x); emb = np.asarray(emb)
    W_emb = np.asarray(W_emb); b_emb = np.asarray(b_emb)
    W_proj = np.asarray(W_proj); W_out = np.asarray(W_out)

    key = ("nc", _debug)
    if key not in _cached:
        _cached[key] = build_kernel(debug=_debug)
    nc = _cached[key]

    in_maps = _prep_host(x, emb, W_emb, b_emb, W_proj, W_out)
    res = run_bass_kernel_spmd(nc, in_maps, core_ids=list(range(NCORES)), trace=_trace,
                               tmpdir=_tmpdir)
    _cached["last_result"] = res

    outp = np.empty((B, N, C), dtype=np.float32)
    for c in range(NCORES):
        b, j = c // 2, c % 2
        outp[b, j * TQ:(j + 1) * TQ, :] = res.results[c]["out"]
    return outp


# revision 37
# speedup vs baseline: 1.2400x; 1.0043x over previous
"""AttentionBlock Trainium2 kernel (B=4, N=2048, C=1024, H=16, D=64, EMB=1024).

    se = emb @ W_emb.T + b_emb;  scale, shift = split(se, 2, -1)
    h  = LN(x) * (1+scale) + shift
    q,k,v = split(h @ W_proj.T) -> (B,H,N,D);  q = LN(q); k = LN(k)  (over D)
    o  = softmax(q k^T / sqrt(D)) v  -> (B,N,C)
    out = o + o @ W_out.T

Sharding: 8 cores; core c -> batch b=c//2, query-half j=c%2. The host rolls
the token axis per core so its query tokens are always tokens 0:1024. Each
core computes the preamble (se/h/qkv/LN) for ONLY its 1024 tokens; centered
keys (kc) and values are then AllGathered across the core pair ([0,1],[2,3],
[4,5],[6,7]) in rank-major key order (attention is permutation-equivariant
over keys), and each core runs attention + out-proj for its 1024 query rows.

Dataflow is feature-major (channels on partitions) end to end; all matmul
operands are bf16 (PSUM accumulation stays fp32):
  - LayerNorm over channels == partition reduction -> ones-column matmuls;
    per-token (free-dim) row stats broadcast across partitions with gpsimd
    partition_broadcast (no DRAM bounce). partition_broadcast only reads a
    tile's partition 0 and only writes base-0 dests, so head-B rows hop to
    partition 0 via a tiny SBUF->SBUF DMA, and two-half tiles are built as
    full-tile broadcast (B) + top-half overwrite (A).
  - q/k LN is folded in *before* the score matmul: kc = (k - mu_k)*rk and
    qc = (q - mu_q)*rq in stacked [128, T] layouts (head pair on partition
    halves), so scores are two concurrent row-tiled K=64 matmuls (tile rows
    0-63 / 64-127) and the softmax exp needs no per-partition scale: one
    ACTIVATE covers both heads' PSUM banks with the 1/sqrt(D) folded into
    its affine scale (0.125).
  - Per-head per-token mean/E[x^2] come from [P,33]-column reducer matmuls
    (head A -> partition 0, head B -> partition 32, both 32-aligned for
    engine slicing); transforms run on the full [33,*] rows (DVE cost is
    per-partition-free-size, the garbage rows are free).
  - rstd/rk/rq = exp(-0.5*ln(var+eps)); a manually emitted InstLoadActFuncSet
    of the natural_log_exp_and_others set keeps the scalar engine on ONE
    table set for the whole kernel (no thrash against the softmax Exp).
  - Softmax denominators come free as a ones column appended to v (row 64 of
    the attnv accumulator); division is fused into the o evacuation via
    reciprocal_approx_fast + gpsimd broadcast.
  - The residual is folded into the output projection: W_res = (I+W_out).T,
    prefetched at kernel start so phase C never stalls.
"""

import sys

sys.path.insert(0, "/opt/trn_rl_repo")

import numpy as np
import ml_dtypes

import concourse.bass as bass
import concourse.mybir as mybir
import concourse.tile as tile
from concourse import bacc
from concourse.bass_utils import run_bass_kernel_spmd

P = 128
B, N, C = 4, 2048, 1024
H, D = 16, 64
EMB = 1024
EPS = 1e-5
T = N          # tokens per batch on each core (k/v coverage)
TQ = N // 2    # query tokens per core
CH = C // P    # 8 channel chunks
O2 = 2 * C
NCORES = 8
TT = 512       # token tile in phase A1
NTT = TQ // TT   # 2 (h only for own query-half tokens)
NMT = T // 512   # 4  key-token tiles (512)
NMC = T // P     # 16 key-token chunks (128)
NNT = TQ // 512  # 2  query-token tiles (512)
LN8 = float(np.log(8.0))

F32 = mybir.dt.float32
BF16 = mybir.dt.bfloat16
MUL = mybir.AluOpType.mult
ADD = mybir.AluOpType.add
SUB = mybir.AluOpType.subtract
EXP = mybir.ActivationFunctionType.Exp
LN = mybir.ActivationFunctionType.Ln


def _ln_exp_set_id(nc):
    # act_info.json index of the set holding both Ln and Exp
    from concourse.hw_specs import get_activation_tables
    tables = get_activation_tables(nc.m.arch)
    for i, (name, funcs) in enumerate(tables.items()):
        if LN in funcs and EXP in funcs:
            return i
    raise RuntimeError("no ACT table set with both Ln and Exp")

_cached = {}


def build_kernel(debug=None):
    nc = bacc.Bacc(num_devices=NCORES)

    xT = nc.dram_tensor("xT", [C, T], BF16, kind="ExternalInput")
    embT = nc.dram_tensor("embT", [EMB, T], BF16, kind="ExternalInput")
    WembT = nc.dram_tensor("WembT", [EMB, O2], BF16, kind="ExternalInput")
    bemb = nc.dram_tensor("bemb", [P, O2 // P], F32, kind="ExternalInput")
    WprojT = nc.dram_tensor("WprojT", [C, 3 * C], BF16, kind="ExternalInput")
    WresT = nc.dram_tensor("WresT", [C, C], BF16, kind="ExternalInput")
    out = nc.dram_tensor("out", [TQ, C], F32, kind="ExternalOutput")

    RGROUPS = [[0, 1], [2, 3], [4, 5], [6, 7]]
    xT_r = xT.rearrange("(ch p) t -> p ch t", p=P)
    embT_r = embT.rearrange("(ch p) t -> p ch t", p=P)
    WembT_r = WembT.rearrange("(ch p) o -> p ch o", p=P)
    WprojT_r = WprojT.rearrange("(ch p) o -> p ch o", p=P)
    WresT_r = WresT.rearrange("(ch p) o -> p ch o", p=P)

    with tile.TileContext(nc) as tc:
        with (
            tc.tile_pool(name="const", bufs=1) as const,
            tc.tile_pool(name="main", bufs=1) as main,
            tc.tile_pool(name="dram", bufs=2, space="DRAM") as dram,
        ):
            # ---------------- constants ----------------
            eps_t = const.tile([P, 1], F32, name="eps_t")
            nc.vector.memset(eps_t[:], EPS)
            nln8_t = const.tile([P, 1], F32, name="nln8_t")
            nc.vector.memset(nln8_t[:], -LN8)
            ones_col = const.tile([P, 1], BF16, name="ones_col")
            nc.vector.memset(ones_col[:], 1.0)
            # preload the combined ln+exp ACT table set so the stats' Ln/Exp
            # never thrash tables against the softmax Exp stream
            nc.scalar.add_instruction(mybir.InstLoadActFuncSet(
                name=nc.get_next_instruction_name(),
                act_func_set_id=_ln_exp_set_id(nc)))
            # per-head partition-block reducers: head A -> out partition 0,
            # head B -> out partition 32 (32-aligned for engine slicing).
            bo8 = const.tile([P, 33], BF16, name="bo8")    # +1/8 blocks
            bo64 = const.tile([P, 33], BF16, name="bo64")  # +1/64 blocks
            for t_, v_ in ((bo8, 0.125), (bo64, 1.0 / 64)):
                nc.vector.memset(t_[:], 0.0)
                nc.vector.memset(t_[0:64, 0:1], v_)
                nc.vector.memset(t_[64:128, 32:33], v_)
            bemb_sb = const.tile([P, O2 // P], F32, name="bemb_sb")
            nc.sync.dma_start(bemb_sb[:], bemb[:])

            h_sb = main.tile([P, CH, TQ], BF16, name="h_sb")   # 16KB/part (own half)
            o_fm = main.tile([P, CH, TQ], BF16, name="o_fm")   # 16KB/part
            # out-proj weights: prefetched (DMA emitted after A1's loads)
            wres_sb = main.tile([P, CH, C], BF16, name="wres_sb")

            # ============ Phase A1: se + LN(x) + FiLM -> h ============
            with (
                tc.tile_pool(name="wembp", bufs=1) as wembp,
                tc.tile_pool(name="a1s", bufs=2) as a1s,
                tc.tile_pool(name="a1r", bufs=2) as a1r,
                tc.tile_pool(name="ps_se", bufs=6, space="PSUM") as ps_se,
                tc.tile_pool(name="ps_st", bufs=2, space="PSUM") as ps_st,
            ):
                # x/e first (stats can start immediately), then wemb split
                # per output-chunk so the first se chains start early
                xe_t = []
                for tt in range(NTT):
                    tsl = slice(tt * TT, (tt + 1) * TT)
                    x_t = a1s.tile([P, CH, TT], BF16, name="x_t")
                    nc.sync.dma_start(x_t[:], xT_r[:, :, tsl])
                    e_t = a1s.tile([P, CH, TT], BF16, name="e_t")
                    nc.sync.dma_start(e_t[:], embT_r[:, :, tsl])
                    xe_t.append((x_t, e_t))
                wemb_sb = wembp.tile([P, CH, O2], BF16, name="wemb_sb")
                for ch in range(CH):
                    nc.sync.dma_start(wemb_sb[:, :, ch * P:(ch + 1) * P],
                                      WembT_r[:, :, ch * P:(ch + 1) * P])
                    nc.sync.dma_start(wemb_sb[:, :, C + ch * P:C + (ch + 1) * P],
                                      WembT_r[:, :, C + ch * P:C + (ch + 1) * P])
                nc.sync.dma_start(wres_sb[:], WresT_r)

                for tt in range(NTT):
                    tsl = slice(tt * TT, (tt + 1) * TT)
                    x_t, e_t = xe_t[tt]

                    # LN stats over channels (partition reduction via matmul)
                    ps_s = ps_st.tile([1, TT], F32, name="ps_s", tag="ps_st")
                    ps_s2 = ps_st.tile([1, TT], F32, name="ps_s2", tag="ps_st")
                    for ch in range(CH):
                        x2c = a1r.tile([P, TT], BF16, name="x2c", tag="scr")
                        nc.vector.tensor_tensor(x2c[:], x_t[:, ch, :], x_t[:, ch, :], MUL)
                        nc.tensor.matmul(ps_s[:], ones_col[:], x_t[:, ch, :],
                                         start=(ch == 0), stop=(ch == CH - 1))
                        nc.tensor.matmul(ps_s2[:], ones_col[:], x2c[:],
                                         start=(ch == 0), stop=(ch == CH - 1))
                    mu = a1r.tile([1, TT], F32, name="mu")
                    m2 = a1r.tile([1, TT], F32, name="m2")
                    vr = a1r.tile([1, TT], F32, name="vr")
                    rows2 = a1r.tile([1, 2, TT], F32, name="rows2")  # [rstd, nmr]
                    nc.vector.tensor_scalar_mul(mu[:], ps_s[:], 1.0 / C)
                    nc.vector.tensor_tensor(m2[:], mu[:], mu[:], MUL)
                    nc.vector.scalar_tensor_tensor(vr[:], ps_s2[:], 1.0 / C, m2[:],
                                                   MUL, SUB)
                    # rstd = exp(-0.5 * ln(var + eps))
                    nc.scalar.activation(vr[:], vr[:], LN, bias=eps_t[0:1], scale=1.0)
                    nc.scalar.activation(rows2[:, 0, :], vr[:], EXP, bias=0.0, scale=-0.5)
                    nc.vector.scalar_tensor_tensor(rows2[:, 1, :], mu[:], -1.0,
                                                   rows2[:, 0, :], MUL, MUL)
                    bc2 = a1r.tile([P, 2, TT], F32, name="bc2")
                    nc.gpsimd.partition_broadcast(bc2[:], rows2[:])
                    rstd_bc = bc2[:, 0, :]
                    nmr_bc = bc2[:, 1, :]

                    for ch in range(CH):
                        ps_sc = ps_se.tile([P, TT], F32, name="ps_sc", tag="ps_se")
                        for ech in range(CH):
                            nc.tensor.matmul(ps_sc[:],
                                             wemb_sb[:, ech, ch * P:(ch + 1) * P],
                                             e_t[:, ech, :],
                                             start=(ech == 0), stop=(ech == CH - 1))
                        ps_sh = ps_se.tile([P, TT], F32, name="ps_sh", tag="ps_se")
                        for ech in range(CH):
                            nc.tensor.matmul(ps_sh[:],
                                             wemb_sb[:, ech, C + ch * P:C + (ch + 1) * P],
                                             e_t[:, ech, :],
                                             start=(ech == 0), stop=(ech == CH - 1))
                        xn = a1r.tile([P, TT], F32, name="xn", tag="scr2")
                        tm = a1r.tile([P, TT], F32, name="tm", tag="scr3")
                        nc.vector.tensor_tensor(xn[:], x_t[:, ch, :], rstd_bc, MUL)
                        nc.vector.tensor_tensor(xn[:], xn[:], nmr_bc, ADD)
                        # (sc + bemb_sc) * xn   (bemb has the FiLM +1 folded in)
                        nc.vector.scalar_tensor_tensor(tm[:], ps_sc[:],
                                                       bemb_sb[:, ch:ch + 1], xn[:],
                                                       ADD, MUL)
                        nc.vector.scalar_tensor_tensor(h_sb[:, ch, tsl], ps_sh[:],
                                                       bemb_sb[:, CH + ch:CH + ch + 1],
                                                       tm[:], ADD, ADD)

            if debug == "h":
                nc.gpsimd.dma_start(out.rearrange("(ch p) t -> p ch t", p=P),
                                    h_sb[:, :, 0:TQ])
            if debug != "h":
                # ============ Phase B: per-head-pair qkv + attention ============
                with (
                    tc.tile_pool(name="bw", bufs=3) as bw,
                    tc.tile_pool(name="batt", bufs=1) as batt,
                    tc.tile_pool(name="bk", bufs=2) as bk,
                    tc.tile_pool(name="bp", bufs=6) as bp,
                    tc.tile_pool(name="bsm", bufs=3) as bsm,
                    tc.tile_pool(name="ps_mm", bufs=2, space="PSUM") as ps_mm,
                    tc.tile_pool(name="ps_sc", bufs=2, space="PSUM") as ps_sc_p,
                    tc.tile_pool(name="ps_ot", bufs=2, space="PSUM") as ps_ot,
                ):
                    for hq in range(4):  # head quads
                        wv_sb = bw.tile([P, CH, 256], BF16, name="wv_sb")
                        nc.sync.dma_start(wv_sb[:], WprojT_r[:, :, 2 * C + hq * 256:2 * C + (hq + 1) * 256])
                        v_sb = batt.tile([P, NMC, 4, 72], BF16, name="v_sb")
                        nc.vector.memset(v_sb[:, :, :, 64:65], 1.0)
                        # own-half v -> DRAM -> pair AllGather -> all 16 chunks
                        vj_d = dram.tile([TQ, 256], BF16, name="vj_d")
                        vg_d = dram.tile([2, TQ, 256], BF16, name="vg_d")
                        for mtk in range(NMC // 2):
                            ps_v = ps_mm.tile([P, 4, 64], F32, name="ps_v", tag="ps_mm")
                            for ch in range(CH):
                                nc.tensor.matmul(ps_v[:], h_sb[:, ch, mtk * P:(mtk + 1) * P],
                                                 wv_sb[:, ch, :], start=(ch == 0), stop=(ch == CH - 1))
                            vtmp = bsm.tile([P, 256], BF16, name="vtmp")
                            nc.vector.tensor_copy(vtmp[:], ps_v[:])
                            nc.sync.dma_start(vj_d[mtk * P:(mtk + 1) * P, :], vtmp[:])
                        nc.gpsimd.collective_compute(
                            "AllGather", mybir.AluOpType.bypass,
                            replica_groups=RGROUPS,
                            ins=[vj_d[:]], outs=[vg_d[:]])
                        for r in range(2):
                            for i in range(NMC // 2):
                                nc.sync.dma_start(
                                    v_sb[:, r * 8 + i, :, 0:64],
                                    vg_d[r, i * P:(i + 1) * P, :].rearrange(
                                        "p (hh d) -> p hh d", hh=4))

                        for hp in (2 * hq, 2 * hq + 1):
                            wqk_sb = bw.tile([P, CH, 256], BF16, name="wqk_sb")
                            nc.sync.dma_start(wqk_sb[:, :, 0:128], WprojT_r[:, :, hp * P:(hp + 1) * P])
                            nc.sync.dma_start(wqk_sb[:, :, 128:256],
                                              WprojT_r[:, :, C + hp * P:C + (hp + 1) * P])

                            # ---- k/q projection + row-space stats (own half) ----
                            # scl33 free layout: [2 (rk|mu), 4 (2 k-mts + 2 q-nts), 512]
                            # rows: head A at partition 0, head B at partition 32
                            NMTO = NMT // 2  # own-half k tiles
                            NCK = NMTO + NNT  # 4 chunks per hp
                            k2sb = bk.tile([P, NMTO, 512], BF16, name="k2sb")
                            q2sb = bk.tile([P, NNT, 512], BF16, name="q2sb")
                            scl33 = bk.tile([33, 2, NCK, 512], BF16, name="scl33")
                            v33 = bk.tile([33, NCK, 512], F32, name="v33")
                            for ck in range(NCK):
                                is_k = ck < NMTO
                                csl = slice(ck * 512, (ck + 1) * 512) if is_k else \
                                    slice((ck - NMTO) * 512, (ck - NMTO + 1) * 512)
                                wsl = slice(128, 256) if is_k else slice(0, 128)
                                dst = k2sb[:, ck, :] if is_k else q2sb[:, ck - NMTO, :]
                                ps_k = ps_mm.tile([P, 512], F32, name="ps_k", tag="ps_mm")
                                for ch in range(CH):
                                    nc.tensor.matmul(ps_k[:], wqk_sb[:, ch, wsl],
                                                     h_sb[:, ch, csl],
                                                     start=(ch == 0), stop=(ch == CH - 1))
                                nc.vector.tensor_copy(dst, ps_k[:])
                                ksq = bsm.tile([P, 512], BF16, name="ksq")
                                nc.vector.tensor_tensor(ksq[:], dst, dst, MUL)
                                ps_kr = ps_mm.tile([33, 512], F32, name="ps_kr", tag="ps_mm")
                                nc.tensor.matmul(ps_kr[:], bo8[:], dst, start=True, stop=True)
                                ps_kr2 = ps_mm.tile([33, 512], F32, name="ps_kr2", tag="ps_mm")
                                nc.tensor.matmul(ps_kr2[:], bo64[:], ksq[:], start=True, stop=True)
                                # mu = (8 mu)/8 ; var = E[k^2] - mu^2
                                nc.vector.tensor_scalar_mul(scl33[:, 1, ck, :], ps_kr[:], 0.125)
                                m2r = bsm.tile([33, 512], F32, name="m2r")
                                nc.vector.tensor_tensor(m2r[:], scl33[:, 1, ck, :],
                                                        scl33[:, 1, ck, :], MUL)
                                nc.vector.tensor_tensor(v33[:, ck, :], ps_kr2[:], m2r[:], SUB)
                            # batched rsqrt: rk = exp(-0.5 ln(var+eps))
                            nc.scalar.activation(v33[:], v33[:], LN, bias=eps_t[0:33], scale=1.0)
                            nc.scalar.activation(scl33[:, 0, :, :], v33[:], EXP, bias=0.0, scale=-0.5)

                            # kc/qc = (kq - mu) * rk, stacked [128, *] bf16
                            kco = bk.tile([P, NMTO, 512], BF16, name="kco")
                            kc = bk.tile([P, NMT, 512], BF16, name="kc")
                            qc = bk.tile([P, NNT, 512], BF16, name="qc")
                            for ck in range(NCK):
                                is_k = ck < NMTO
                                src = kco[:, ck, :] if is_k else q2sb[:, ck - NMTO, :]
                                src = k2sb[:, ck, :] if is_k else q2sb[:, ck - NMTO, :]
                                dst = kco[:, ck, :] if is_k else qc[:, ck - NMTO, :]
                                sclB0 = bsm.tile([1, 2, 512], BF16, name="sclB0")
                                nc.sync.dma_start(sclB0[:], scl33[32:33, :, ck, :])
                                bc = bsm.tile([P, 2, 512], BF16, name="bc")
                                # partition_broadcast only writes from a tile's
                                # partition 0: fill all 128 with head B, then
                                # overwrite the top half with head A.
                                nc.gpsimd.partition_broadcast(bc[:], sclB0[:])
                                nc.gpsimd.partition_broadcast(bc[0:64, :, :],
                                                              scl33[0:1, :, ck, :])
                                kct = bsm.tile([P, 512], F32, name="kct")
                                nc.vector.tensor_tensor(kct[:], src, bc[:, 1, :], SUB)
                                nc.vector.tensor_tensor(dst, kct[:], bc[:, 0, :], MUL)
                            # gather centered k across the core pair
                            kcj_d = dram.tile([P, TQ], BF16, name="kcj_d")
                            kcg_d = dram.tile([2, P, TQ], BF16, name="kcg_d")
                            nc.sync.dma_start(kcj_d[:], kco[:])
                            nc.gpsimd.collective_compute(
                                "AllGather", mybir.AluOpType.bypass,
                                replica_groups=RGROUPS,
                                ins=[kcj_d[:]], outs=[kcg_d[:]])
                            for r in range(2):
                                nc.sync.dma_start(kc[:, 2 * r:2 * r + 2, :],
                                                  kcg_d[r].rearrange("p (a b) -> p a b", a=NMTO))

                            if debug == "qa":
                                ofl = out.rearrange("(a b) t -> a (b t)", a=P)
                                nc.gpsimd.dma_start(ofl[:, 0:TQ], qc[:])
                                nc.gpsimd.dma_start(ofl[:, TQ:TQ + T], kc[:])
                                continue

                            # ---- scores + exp + o per head pair ----
                            hA, hB = 2 * hp, 2 * hp + 1
                            viA = (hp % 2) * 2
                            viB = viA + 1
                            for nt in range(NNT):
                                nsl = slice(nt * 512, (nt + 1) * 512)
                                ps_oA = ps_ot.tile([65, 512], F32, name="ps_oA", tag="ps_ot")
                                ps_oB = ps_ot.tile([65, 512], F32, name="ps_oB", tag="ps_ot")
                                for mc in range(NMC):
                                    mt, off = mc // 4, (mc % 4) * P
                                    ps_s = ps_sc_p.tile([P, 2, 512], F32, name="ps_s", tag="ps_sc")
                                    nc.tensor.matmul(ps_s[:, 0, :],
                                                     kc[0:64, mt, off:off + P],
                                                     qc[0:64, nt, :], start=True, stop=True)
                                    nc.tensor.matmul(ps_s[:, 1, :],
                                                     kc[64:128, mt, off:off + P],
                                                     qc[64:128, nt, :], start=True, stop=True)
                                    p_t = bp.tile([P, 2, 512], BF16, name="p_t")
                                    nc.scalar.activation(p_t[:], ps_s[:], EXP, bias=0.0, scale=0.125)
                                    nc.tensor.matmul(ps_oA[:], v_sb[:, mc, viA, 0:65],
                                                     p_t[:, 0, :],
                                                     start=(mc == 0), stop=(mc == NMC - 1))
                                    nc.tensor.matmul(ps_oB[:], v_sb[:, mc, viB, 0:65],
                                                     p_t[:, 1, :],
                                                     start=(mc == 0), stop=(mc == NMC - 1))
                                # softmax division fused into the evacuation:
                                # den row hops to partition 0 (aligned -64),
                                # reciprocal, gpsimd-broadcast, then
                                # o_fm = ps_o[0:64] * rec_bc in one DVE op.
                                for ps_oX, hX in ((ps_oA, hA), (ps_oB, hB)):
                                    den_row = bsm.tile([1, 512], F32, name="den_row")
                                    nc.vector.tensor_copy(den_row[:], ps_oX[64:65, :])
                                    nc.vector.reciprocal_approx_fast(den_row[:], den_row[:])
                                    dbc = bsm.tile([64, 512], F32, name="dbc")
                                    nc.gpsimd.partition_broadcast(dbc[:], den_row[:])
                                    nc.vector.tensor_tensor(
                                        o_fm[(hX % 2) * 64:(hX % 2) * 64 + 64, hX // 2, nsl],
                                        ps_oX[0:64, :], dbc[:], MUL)

                if debug == "b1":
                    nc.gpsimd.dma_start(out.rearrange("(ch p) t -> p ch t", p=P),
                                        o_fm[:, :, :])
                # ============ Phase C: out = o_fm.T @ (I + W_out).T ============
                if debug is None:
                  with (
                      tc.tile_pool(name="cw", bufs=2) as cw,
                      tc.tile_pool(name="ps_c", bufs=2, space="PSUM") as ps_c,
                  ):
                    for jt in range(C // 512):
                        jsl = slice(jt * 512, (jt + 1) * 512)
                        for ns in range(TQ // P):
                            ps_f = ps_c.tile([P, 512], F32, name="ps_f", tag="ps_c")
                            for cg in range(CH):
                                nc.tensor.matmul(ps_f[:], o_fm[:, cg, ns * P:(ns + 1) * P],
                                                 wres_sb[:, cg, jsl],
                                                 start=(cg == 0), stop=(cg == CH - 1))
                            f_sb = cw.tile([P, 512], F32, name="f_sb")
                            nc.vector.tensor_copy(f_sb[:], ps_f[:])
                            nc.sync.dma_start(out[ns * P:(ns + 1) * P, jt * 512:(jt + 1) * 512],
                                              f_sb[:])

    nc.finalize()
    return nc


def _prep_host(x, emb, W_emb, b_emb, W_proj, W_out):
    bf16 = ml_dtypes.bfloat16
    W_embT = np.ascontiguousarray(W_emb.T.astype(bf16))
    W_projT = np.ascontiguousarray(W_proj.T.astype(bf16))
    W_resT = np.ascontiguousarray((np.eye(C, dtype=np.float32) + W_out).T.astype(bf16))
    bemb2 = b_emb.astype(np.float32).copy()
    bemb2[:C] += 1.0                       # fold the FiLM "+1" into the bias
    bemb_col = np.ascontiguousarray(bemb2.reshape(O2 // P, P).T)

    in_maps = []
    for c in range(NCORES):
        b, j = c // 2, c % 2
        perm = np.concatenate([np.arange(j * TQ, (j + 1) * TQ),
                               np.arange((1 - j) * TQ, (2 - j) * TQ)])
        in_maps.append({
            "xT": np.ascontiguousarray(x[b][perm].T.astype(bf16)),
            "embT": np.ascontiguousarray(emb[b][perm].T.astype(bf16)),
            "WembT": W_embT, "bemb": bemb_col,
            "WprojT": W_projT, "WresT": W_resT,
        })
    return in_maps


def kernel(x, emb, W_emb, b_emb, W_proj, W_out, _trace=False, _tmpdir=None, _debug=None):
    x = np.asarray(x); emb = np.asarray(emb)
    W_emb = np.asarray(W_emb); b_emb = np.asarray(b_emb)
    W_proj = np.asarray(W_proj); W_out = np.asarray(W_out)

    key = ("nc", _debug)
    if key not in _cached:
        _cached[key] = build_kernel(debug=_debug)
    nc = _cached[key]

    in_maps = _prep_host(x, emb, W_emb, b_emb, W_proj, W_out)
    res = run_bass_kernel_spmd(nc, in_maps, core_ids=list(range(NCORES)), trace=_trace,
                               tmpdir=_tmpdir)
    _cached["last_result"] = res

    outp = np.empty((B, N, C), dtype=np.float32)
    for c in range(NCORES):
        b, j = c // 2, c % 2
        outp[b, j * TQ:(j + 1) * TQ, :] = res.results[c]["out"]
    return outp
